# revision 28
# baseline (speedup 1.0000x reference)
"""Trainium2 Bass kernel for DAResBlock3D (dual-attention residual block).

Strategy (8 NeuronCores, SPMD):
  - Spatial sharding over H: core i owns output h-slabs {2i, 2i+1} (512 of
    4096 positions per batch), both batches on-chip as partition halves.
  - 3x3x3 convs: 27 shifted matmuls over a zero-padded local view (4 h-slabs
    with halo), with 2-way PE packing: row groups = batch.
  - BatchNorm (train-mode, global stats): per-core partial sums AllGathered
    (1KB) and reduced redundantly on every core.
  - PAM: energy computed transposed (E^T tiles, m on partitions); softmax
    without max-subtraction (energies are small); exp on ScalarE in
    (128,1024) chunks; O = v @ A^T via augmented v^T (ones column gives the
    softmax denominator for free).
  - CAM: per-core partial Gram (64x64) AllGathered; softmax redundant.
  - Host<->device traffic minimized (the wall-clock dispatch cost is
    transfer-dominated): conv + qkv weights ship as a 1/8 shard per core and
    are AllGathered on device; x ships as each core's own 2 h-slabs only
    (compact, bf16) with halo slabs exchanged on-device; output returns bf16.
"""

import os
import sys

sys.path.insert(0, "/opt/trn_rl_repo")

import numpy as np

import jax

jax.config.update("jax_compilation_cache_dir", "/tmp/jax_bass_cc_cache")
jax.config.update("jax_persistent_cache_min_compile_time_secs", 0.0)
jax.config.update("jax_persistent_cache_min_entry_size_bytes", -1)

import concourse.bass as bass
import concourse.mybir as mybir
import concourse.tile as tile
from concourse import bacc
from concourse.bass_utils import run_bass_kernel_spmd
from concourse.masks import make_identity

F32 = mybir.dt.float32
F32R = mybir.dt.float32r
BF16 = mybir.dt.bfloat16
U32 = mybir.dt.uint32
U16 = mybir.dt.uint16
U8 = mybir.dt.uint8
AF = mybir.ActivationFunctionType
ALU = mybir.AluOpType
AX = mybir.AxisListType

NCORES = 8
B = 2
C = 64
HH = 16
N = HH * HH * HH  # 4096
ROW = 18 * 18  # 324, one padded h-slab (w,d padded to 18x18)
LOCPAD = 19  # only w/d deltas (+-18, +-1) can underflow a slab base
LOCVIEW = LOCPAD + 4 * ROW + LOCPAD  # local act view: 4 h-slabs + margins
SLAB = 256  # interior positions per h-slab (16x16)
SHARD = 2 * SLAB  # 512 interior positions per batch per core
SLOPE = (1.0 / 8.0 + 1.0 / 3.0) / 2.0  # RReLU eval negative slope
EPS = 1e-5
NTOT = B * N  # BN normalization count = 8192

# weight blob geometry (full blob is [WB_ROWS, WB_W] bf16, sharded 1/8 per
# core along rows and AllGathered on device)
WB_W = 27 * 64  # 1728
WB_ROWS = 392  # 4*64 (convS,C,S1,C1) + 128 (convF) + 8 (qkv flat)
WB_SH = WB_ROWS // NCORES  # 49 rows per core
QKV_OFF = 384 * WB_W  # flat elem offset of the qkv region

AG2_S1 = 2 * B * C * SLAB  # 65536: s1 region elems per rank
AG2_GRAM = B * C * C  # 8192: gram region elems per rank
AG2_PER = AG2_S1 + AG2_GRAM  # 73728

# packed per-core input blob (bf16 carrier elems). x and w ship as separate
# hi/lo byte planes: the tunnel zstd-compresses the wire, and the hi byte
# (sign+exponent) of random bf16 data is low-entropy while interleaved
# bf16 is not. Planes are reassembled on device with integer shifts.
XN = 2 * B * C * SLAB  # 65536 x elems per core (own 2 slabs)
WN = WB_SH * WB_W  # 84672 weight-shard elems per core
BLOB_BN = 64 * 12  # 768 bf16 bn/gamma params
OFF_XHI = 0  # x hi-plane: XN bytes = XN/2 carrier elems
OFF_XLO = XN // 2
OFF_WHI = XN
OFF_WLO = XN + WN // 2
OFF_BN = XN + WN
BLOB_TOT = XN + WN + BLOB_BN  # 150976


def _deltas():
    out = []
    for dh in (-1, 0, 1):
        for dw in (-1, 0, 1):
            for dd in (-1, 0, 1):
                out.append(dh * ROW + dw * 18 + dd)
    return out


DELTAS = _deltas()


def build_program():
    nc = bacc.Bacc(
        "TRN2",
        target_bir_lowering=False,
        debug=False,
        num_devices=NCORES,
    )

    # ---- external input: one packed blob per core (x2 | wblob shard | bnp)
    blob_d = nc.dram_tensor("blob", [BLOB_TOT], BF16, kind="ExternalInput")
    # output ships as hi/lo byte planes too (bf16 carrier, reassembled host-side)
    out_d = nc.dram_tensor(
        "out", [2, B * C * SHARD // 2], BF16, kind="ExternalOutput"
    )

    rg = [list(range(NCORES))]

    with tile.TileContext(nc) as tc:
        dram_cm = tc.tile_pool(name="dram", bufs=1, space="DRAM")
        dram = dram_cm.__enter__()
        # weight blob gather
        wb_in = dram.tile([WB_SH, WB_W], BF16)
        wb_ag = dram.tile([NCORES, WB_SH, WB_W], BF16, addr_space="Shared")
        # x halo exchange buffers (own slabs shipped; halos fetched on-device)
        x2_in = dram.tile([2, B, C, SLAB], BF16)
        x2_ag = dram.tile([NCORES, 2, B, C, SLAB], BF16, addr_space="Shared")
        x2_ri = dram.tile([NCORES, 2, B, C, SLAB], BF16)
        x2_ro = dram.tile([2, B, C, SLAB], BF16)
        # collective bounce buffers
        st1_in = dram.tile([64, 4], F32)
        st1_out = dram.tile([NCORES, 64, 4], F32, addr_space="Shared")
        ag2_in = dram.tile([AG2_PER], F32)
        ag2_out = dram.tile([NCORES * AG2_PER], F32, addr_space="Shared")
        c2_in = dram.tile([2, B, C, SLAB], BF16)
        c2_ag = dram.tile([NCORES, 2, B, C, SLAB], BF16, addr_space="Shared")
        c2_ri = dram.tile([NCORES, 2, B, C, SLAB], BF16)
        c2_ro = dram.tile([2, B, C, SLAB], BF16)
        s2_in = dram.tile([2, B, C, SLAB], BF16)
        s2_ag = dram.tile([NCORES, 2, B, C, SLAB], BF16, addr_space="Shared")
        s2_ri = dram.tile([NCORES, 2, B, C, SLAB], BF16)
        s2_ro = dram.tile([2, B, C, SLAB], BF16)
        st2_in = dram.tile([64, 4], F32)
        st2_out = dram.tile([NCORES, 64, 4], F32, addr_space="Shared")
        fc_in = dram.tile([2, B, 2 * C, SLAB], BF16)
        fc_ag = dram.tile([NCORES, 2, B, 2 * C, SLAB], BF16, addr_space="Shared")
        fc_ri = dram.tile([NCORES, 2, B, 2 * C, SLAB], BF16)
        fc_ro = dram.tile([2, B, 2 * C, SLAB], BF16)
        stf_in = dram.tile([64, 2], F32)
        stf_out = dram.tile([NCORES, 64, 2], F32, addr_space="Shared")
        bcast_dram = dram.tile([B, SHARD], F32)

        singles_cm = tc.tile_pool(name="singles", bufs=1)
        singles = singles_cm.__enter__()

        ident = singles.tile([64, 64], BF16)
        make_identity(nc, ident[:])
        ident_f32 = singles.tile([64, 64], F32)
        make_identity(nc, ident_f32[:])

        # gather the weight blob first; everything weight-like reads from it
        # reassemble the weight shard from hi/lo byte planes, then stage to
        # DRAM for the AllGather (96x882 == 49x1728 flat)
        whi8 = singles.tile([96, 882], U8)
        wlo8 = singles.tile([96, 882], U8)
        nc.sync.dma_start(
            out=whi8[:],
            in_=bass.AP(
                tensor=blob_d, offset=OFF_WHI, ap=[[441, 96], [1, 441]]
            ).bitcast(U8),
        )
        nc.sync.dma_start(
            out=wlo8[:],
            in_=bass.AP(
                tensor=blob_d, offset=OFF_WLO, ap=[[441, 96], [1, 441]]
            ).bitcast(U8),
        )
        whi16 = singles.tile([96, 882], U16)
        wlo16 = singles.tile([96, 882], U16)
        nc.vector.tensor_copy(whi16[:], whi8[:])
        nc.vector.tensor_copy(wlo16[:], wlo8[:])
        nc.vector.tensor_scalar(
            whi16[:], whi16[:], 8, None, ALU.logical_shift_left
        )
        wres = singles.tile([96, 882], BF16)
        nc.vector.tensor_tensor(
            wres[:].bitcast(U16), whi16[:], wlo16[:], ALU.bitwise_or
        )
        nc.sync.dma_start(out=wb_in[:], in_=wres[:])
        nc.gpsimd.collective_compute(
            "AllGather",
            ALU.bypass,
            replica_groups=rg,
            ins=[wb_in[:].opt()],
            outs=[wb_ag[:].opt()],
        )

        # constants to SBUF
        qw_sb = singles.tile([65, 64], BF16)
        kw_sb = singles.tile([65, 64], BF16)
        vw_sb = singles.tile([65, 66], BF16)
        bnp_bf = singles.tile([64, 12], BF16)
        bnp = singles.tile([64, 12], F32)
        nc.sync.dma_start(
            out=qw_sb[:],
            in_=bass.AP(
                tensor=wb_ag[:].tensor, offset=QKV_OFF, ap=[[64, 65], [1, 64]]
            ),
        )
        nc.sync.dma_start(
            out=kw_sb[:],
            in_=bass.AP(
                tensor=wb_ag[:].tensor,
                offset=QKV_OFF + 65 * 64,
                ap=[[64, 65], [1, 64]],
            ),
        )
        nc.sync.dma_start(
            out=vw_sb[:],
            in_=bass.AP(
                tensor=wb_ag[:].tensor,
                offset=QKV_OFF + 2 * 65 * 64,
                ap=[[66, 65], [1, 66]],
            ),
        )
        nc.sync.dma_start(
            out=bnp_bf[:],
            in_=bass.AP(
                tensor=blob_d, offset=OFF_BN, ap=[[12, 64], [1, 12]]
            ),
        )
        nc.vector.tensor_copy(bnp[:], bnp_bf[:])
        ones_row = singles.tile([1, 64], F32)
        nc.vector.memset(ones_row[:], 1.0)
        eps_col = singles.tile([64, 1], F32)
        nc.vector.memset(eps_col[:], EPS)
        zrow = singles.tile([128, SLAB], BF16)
        nc.vector.memset(zrow[:], 0.0)

        # big persistent activations
        acts_cm = tc.tile_pool(name="acts", bufs=1)
        acts = acts_cm.__enter__()

        # stage own x slabs from hi/lo byte planes; halos fetched on-device
        # (phase 0 below). Rows p=(b,c), cols (j,s).
        xhi8 = acts.tile([128, SHARD], U8)
        xlo8 = acts.tile([128, SHARD], U8)
        nc.sync.dma_start(
            out=xhi8[:],
            in_=bass.AP(
                tensor=blob_d, offset=OFF_XHI, ap=[[SHARD // 2, 128], [1, SHARD // 2]]
            ).bitcast(U8),
        )
        nc.sync.dma_start(
            out=xlo8[:],
            in_=bass.AP(
                tensor=blob_d, offset=OFF_XLO, ap=[[SHARD // 2, 128], [1, SHARD // 2]]
            ).bitcast(U8),
        )
        xhi16 = acts.tile([128, SHARD], U16)
        xlo16 = acts.tile([128, SHARD], U16)
        nc.vector.tensor_copy(xhi16[:], xhi8[:])
        nc.vector.tensor_copy(xlo16[:], xlo8[:])
        nc.vector.tensor_scalar(
            xhi16[:], xhi16[:], 8, None, ALU.logical_shift_left
        )
        x2_sb = acts.tile([128, SHARD], BF16)
        nc.vector.tensor_tensor(
            x2_sb[:].bitcast(U16), xhi16[:], xlo16[:], ALU.bitwise_or
        )
        x_sb = acts.tile([128, LOCVIEW], BF16)
        nc.gpsimd.memset(x_sb[:], 0.0)

        s1_own = [acts.tile([65, SHARD], F32, name=f"s1own{b}") for b in range(B)]
        s1_own_bf = [acts.tile([65, SHARD], BF16, name=f"s1ownbf{b}") for b in range(B)]
        c1_own = [acts.tile([64, SHARD], F32, name=f"c1own{b}") for b in range(B)]
        c1_own_bf = [acts.tile([64, SHARD], BF16, name=f"c1ownbf{b}") for b in range(B)]
        for b in range(B):
            nc.vector.memset(s1_own[b][64:65, :], 1.0)
            nc.vector.memset(s1_own_bf[b][64:65, :], 1.0)

        s1_pam = [acts.tile([65, N], BF16, name=f"s1pam{b}") for b in range(B)]
        for b in range(B):
            nc.vector.memset(s1_pam[b][64:65, :], 1.0)

        k_stack = acts.tile([128, N], BF16)
        q_stack = acts.tile([128, SHARD], BF16)
        vt_sb = [acts.tile([128, 32 * 66], BF16, name=f"vt{b}") for b in range(B)]

        wpool_cm = tc.tile_pool(name="wpool", bufs=2)
        wpool = wpool_cm.__enter__()

        stats_pool_cm = tc.tile_pool(name="stats", bufs=1)
        stats_pool = stats_pool_cm.__enter__()

        tmp_pool_cm = tc.tile_pool(name="tmp", bufs=2)
        tmp_pool = tmp_pool_cm.__enter__()

        # ---------------- helpers ----------------
        def load_wconv(idx, name):
            """Load conv weights from the gathered blob; idx 0-3 are 64-row
            convs as block-diagonal [128,27,128] (batch packs the PE), idx 4
            is the 128-row fusion conv."""
            if idx < 4:
                w = wpool.tile([128, 27, 128], BF16, tag="wconv", name=name)
                nc.gpsimd.memset(w[:], 0.0)
                base = idx * 64 * WB_W
                for half in range(2):
                    nc.sync.dma_start(
                        out=w[
                            64 * half : 64 * half + 64,
                            :,
                            64 * half : 64 * half + 64,
                        ],
                        in_=bass.AP(
                            tensor=wb_ag[:].tensor,
                            offset=base,
                            ap=[[WB_W, 64], [64, 27], [1, 64]],
                        ),
                    )
            else:
                w = wpool.tile([128, 27, 64], BF16, tag="wconvF", name=name)
                nc.sync.dma_start(
                    out=w[:],
                    in_=bass.AP(
                        tensor=wb_ag[:].tensor,
                        offset=256 * WB_W,
                        ap=[[WB_W, 128], [64, 27], [1, 64]],
                    ),
                )
            return w

        def conv64(w_sb_t, act, psum_pool, tname):
            """3x3x3 conv over 64-ch padded local view for own 2 slabs.

            Block-diagonal weights pack both batches into one 128x128 PE
            matmul per offset. Returns per-batch compact raw-output tiles
            t[b] (64, 512) plus (sum, sumsq) stat columns (64,1) each."""
            touts = [
                stats_pool.tile([64, SHARD], F32, name=f"{tname}_t{b}")
                for b in range(B)
            ]
            for jj, jslab in enumerate((1, 2)):
                ps = psum_pool.tile(
                    [128, ROW], F32, tag="convps", name=f"{tname}ps{jj}"
                )
                base = LOCPAD + jslab * ROW
                for o in range(27):
                    nc.tensor.matmul(
                        ps[:],
                        lhsT=w_sb_t[:, o, :],
                        rhs=act[
                            :, base + DELTAS[o] : base + DELTAS[o] + ROW
                        ],
                        start=(o == 0),
                        stop=(o == 26),
                    )
                for b in range(B):
                    nc.vector.tensor_copy(
                        touts[b][:, jj * SLAB : (jj + 1) * SLAB],
                        ps[64 * b : 64 * b + 64, :]
                        .rearrange("p (w d) -> p w d", w=18)[:, 1:17, 1:17],
                    )
            stats = []
            for b in range(B):
                t = touts[b]
                ssum = stats_pool.tile([64, 1], F32, name=f"{tname}_s{b}")
                ssq = stats_pool.tile([64, 1], F32, name=f"{tname}_q{b}")
                scr2 = tmp_pool.tile([64, SHARD], F32, tag="scrB", name=f"{tname}scrB{b}")
                nc.vector.reduce_sum(ssum[:], t[:], axis=AX.X)
                nc.scalar.activation(scr2[:], t[:], AF.Square, accum_out=ssq[:])
                stats.append((ssum, ssq))
            return touts, stats

        def conv128(w_sb_t, act_pair, psum_pool, tname):
            """3x3x3 conv with 128 input channels (fused concat), per batch."""
            touts = []
            stats = []
            for b in range(B):
                t = stats_pool.tile([64, SHARD], F32, name=f"{tname}_t{b}")
                for jj, jslab in enumerate((1, 2)):
                    ps = psum_pool.tile(
                        [64, ROW], F32, tag=f"convps{b}", name=f"{tname}ps{b}{jj}"
                    )
                    base = LOCPAD + jslab * ROW
                    for o in range(27):
                        nc.tensor.matmul(
                            ps[:],
                            lhsT=w_sb_t[:, o, :],
                            rhs=act_pair[b][
                                :, base + DELTAS[o] : base + DELTAS[o] + ROW
                            ],
                            start=(o == 0),
                            stop=(o == 26),
                        )
                    nc.vector.tensor_copy(
                        t[:, jj * SLAB : (jj + 1) * SLAB],
                        ps[:, :].rearrange("p (w d) -> p w d", w=18)[
                            :, 1:17, 1:17
                        ],
                    )
                touts.append(t)
                ssum = stats_pool.tile([64, 1], F32, name=f"{tname}_s{b}")
                ssq = stats_pool.tile([64, 1], F32, name=f"{tname}_q{b}")
                scr2 = tmp_pool.tile([64, SHARD], F32, tag="scrB", name=f"{tname}scrB{b}")
                nc.vector.reduce_sum(ssum[:], t[:], axis=AX.X)
                nc.scalar.activation(scr2[:], t[:], AF.Square, accum_out=ssq[:])
                stats.append((ssum, ssq))
            return touts, stats

        def pack_stats(dst_sb, stats_list):
            """stats_list: list of (ssum_b0, ssq_b0), (ssum_b1, ssq_b1) pairs
            per conv; writes [sum, sq] per conv into dst columns."""
            for ci, st in enumerate(stats_list):
                (s0, q0), (s1_, q1) = st
                nc.vector.tensor_add(dst_sb[:, 2 * ci : 2 * ci + 1], s0[:], s1_[:])
                nc.vector.tensor_add(
                    dst_sb[:, 2 * ci + 1 : 2 * ci + 2], q0[:], q1[:]
                )

        def bn_coeffs(tot_sb, col, g_col, b_col, name):
            """From total [sum, sumsq] cols compute A=(g*rstd), B=b-mean*A and
            the rrelu-scaled variants. Returns (A, B, As, Bs) (64,1) tiles."""
            mean = stats_pool.tile([64, 1], F32, name=f"{name}_mean")
            var = stats_pool.tile([64, 1], F32, name=f"{name}_var")
            a_t = stats_pool.tile([64, 1], F32, name=f"{name}_A")
            b_t = stats_pool.tile([64, 1], F32, name=f"{name}_B")
            as_t = stats_pool.tile([64, 1], F32, name=f"{name}_As")
            bs_t = stats_pool.tile([64, 1], F32, name=f"{name}_Bs")
            scr = stats_pool.tile([64, 1], F32, name=f"{name}_scr")
            nc.vector.tensor_scalar(
                mean[:], tot_sb[:, col : col + 1], 1.0 / NTOT, None, ALU.mult
            )
            nc.vector.tensor_scalar(
                var[:], tot_sb[:, col + 1 : col + 2], 1.0 / NTOT, None, ALU.mult
            )
            nc.vector.tensor_mul(scr[:], mean[:], mean[:])
            nc.vector.tensor_sub(var[:], var[:], scr[:])
            # rstd = exp(-0.5*ln(var+eps)); avoids the Sqrt table set
            nc.scalar.activation(scr[:], var[:], AF.Ln, bias=eps_col[:])
            nc.vector.tensor_scalar(scr[:], scr[:], -0.5, None, ALU.mult)
            nc.scalar.activation(scr[:], scr[:], AF.Exp)
            nc.vector.tensor_mul(a_t[:], scr[:], g_col)
            nc.vector.tensor_mul(scr[:], mean[:], a_t[:])
            nc.vector.tensor_sub(b_t[:], b_col, scr[:])
            nc.vector.tensor_scalar(as_t[:], a_t[:], SLOPE, None, ALU.mult)
            nc.vector.tensor_scalar(bs_t[:], b_t[:], SLOPE, None, ALU.mult)
            return a_t, b_t, as_t, bs_t

        def bn_rrelu(t_raw, coeffs, dst_ap):
            """dst = max(A*t+B, As*t+Bs) elementwise."""
            a_t, b_t, as_t, bs_t = coeffs
            y1 = tmp_pool.tile([64, SHARD], F32, tag="y1", name="y1_t")
            y2 = tmp_pool.tile([64, SHARD], F32, tag="y2", name="y2_t")
            nc.vector.tensor_scalar(
                y1[:], t_raw[:], a_t[:], b_t[:], ALU.mult, ALU.add
            )
            nc.vector.tensor_scalar(
                y2[:], t_raw[:], as_t[:], bs_t[:], ALU.mult, ALU.add
            )
            nc.vector.tensor_max(dst_ap, y1[:], y2[:])

        def halo_exchange(in_t, ag_t, ri_t, ro_t, nch):
            """AG own slabs, then RS-rotate so each core receives exactly its
            lo/hi halo slabs (slot-static reads of the gathered buffer)."""
            nc.gpsimd.collective_compute(
                "AllGather", ALU.bypass, replica_groups=rg,
                ins=[in_t[:].opt()], outs=[ag_t[:].opt()],
            )
            blk = B * nch * SLAB  # one slab block (elements)
            per = 2 * blk  # one rank contribution
            for i in range(NCORES):
                # lo slot: rank i-1's slab 1
                if i > 0:
                    nc.sync.dma_start(
                        out=bass.AP(
                            tensor=ri_t[:].tensor,
                            offset=i * per,
                            ap=[[1, blk]],
                        ),
                        in_=bass.AP(
                            tensor=ag_t[:].tensor,
                            offset=(i - 1) * per + blk,
                            ap=[[1, blk]],
                        ),
                    )
                else:
                    for z in range(blk // (128 * SLAB)):
                        nc.sync.dma_start(
                            out=bass.AP(
                                tensor=ri_t[:].tensor,
                                offset=z * 128 * SLAB,
                                ap=[[SLAB, 128], [1, SLAB]],
                            ),
                            in_=zrow[:],
                        )
                # hi slot: rank i+1's slab 0
                if i < NCORES - 1:
                    nc.sync.dma_start(
                        out=bass.AP(
                            tensor=ri_t[:].tensor,
                            offset=i * per + blk,
                            ap=[[1, blk]],
                        ),
                        in_=bass.AP(
                            tensor=ag_t[:].tensor,
                            offset=(i + 1) * per,
                            ap=[[1, blk]],
                        ),
                    )
                else:
                    for z in range(blk // (128 * SLAB)):
                        nc.sync.dma_start(
                            out=bass.AP(
                                tensor=ri_t[:].tensor,
                                offset=i * per + blk + z * 128 * SLAB,
                                ap=[[SLAB, 128], [1, SLAB]],
                            ),
                            in_=zrow[:],
                        )
            nc.gpsimd.collective_compute(
                "ReduceScatter", ALU.add, replica_groups=rg,
                ins=[ri_t[:].opt()], outs=[ro_t[:].opt()],
            )

        def build_view(ro_t, nch, bsel, dst, own_ap, name):
            """dst (128, LOCVIEW) bf16: slabs 1-2 <- own; 0/3 <- RS halos/8."""
            blk = B * nch * SLAB
            boff = 0 if bsel is None else bsel * nch * SLAB
            for dslab, hs in ((0, 0), (3, 1)):
                stg = tmp_pool.tile(
                    [128, SLAB], BF16, tag="hstg", name=f"hs{name}{dslab}"
                )
                nc.sync.dma_start(
                    out=stg[:],
                    in_=bass.AP(
                        tensor=ro_t[:].tensor,
                        offset=hs * blk + boff,
                        ap=[[SLAB, 128], [1, SLAB]],
                    ),
                )
                nc.vector.tensor_scalar(
                    dst[:, LOCPAD + dslab * ROW : LOCPAD + (dslab + 1) * ROW]
                    .rearrange("p (w d) -> p w d", w=18)[:, 1:17, 1:17],
                    stg[:].rearrange("p (w d) -> p w d", w=16),
                    1.0 / NCORES,
                    None,
                    ALU.mult,
                )
            nc.vector.tensor_copy(
                dst[:, LOCPAD + 1 * ROW : LOCPAD + 3 * ROW]
                .rearrange("p (j w d) -> p j w d", j=2, w=18)[:, :, 1:17, 1:17],
                own_ap,
            )

        # =========== phase 0: x halo exchange + padded local view ===========
        nc.sync.dma_start(
            out=bass.AP(
                tensor=x2_in[:].tensor,
                offset=0,
                ap=[[SLAB, 128], [B * C * SLAB, 2], [1, SLAB]],
            ),
            in_=x2_sb[:].rearrange("p (j s) -> p j s", j=2),
        )
        halo_exchange(x2_in, x2_ag, x2_ri, x2_ro, C)
        build_view(
            x2_ro, C, None, x_sb,
            x2_sb[:].rearrange("p (j w d) -> p j w d", j=2, w=16), "x2",
        )

        # =========== phase 1: conv S and conv C (input x) ===========
        cpsum_cm = tc.tile_pool(name="cpsum", bufs=2, space="PSUM")
        cpsum = cpsum_cm.__enter__()

        ws_sb = load_wconv(0, "wsS")
        tS, statS = conv64(ws_sb, x_sb, cpsum, "cS")
        wc_sb = load_wconv(1, "wsC")
        tC, statC = conv64(wc_sb, x_sb, cpsum, "cC")

        st1_sb = stats_pool.tile([64, 4], F32)
        pack_stats(st1_sb, [statS, statC])
        nc.sync.dma_start(out=st1_in[:], in_=st1_sb[:])
        nc.gpsimd.collective_compute(
            "AllGather",
            ALU.bypass,
            replica_groups=rg,
            ins=[st1_in[:].opt()],
            outs=[st1_out[:].opt()],
        )

        # reduce gathered stats and compute BN coefficients
        st1_stage = stats_pool.tile([64, 4, NCORES], F32)
        nc.sync.dma_start(
            out=st1_stage[:],
            in_=bass.AP(
                tensor=st1_out[:].tensor,
                offset=0,
                ap=[[4, 64], [1, 4], [256, NCORES]],
            ),
        )
        st1_tot = stats_pool.tile([64, 4], F32)
        nc.vector.tensor_reduce(st1_tot[:], st1_stage[:], axis=AX.X, op=ALU.add)
        cS = bn_coeffs(st1_tot, 0, bnp[:, 0:1], bnp[:, 1:2], "bnS")
        cC = bn_coeffs(st1_tot, 2, bnp[:, 2:3], bnp[:, 3:4], "bnC")

        for b in range(B):
            bn_rrelu(tS[b], cS, s1_own[b][0:64, :])
            bn_rrelu(tC[b], cC, c1_own[b][:, :])
            nc.vector.tensor_copy(s1_own_bf[b][0:64, :], s1_own[b][0:64, :])
            nc.vector.tensor_copy(c1_own_bf[b][:, :], c1_own[b][:, :])

        cpsum_cm.__exit__(None, None, None)

        # =========== phase 2: CAM partial gram + AG2 (s1 + gram) ===========
        mpsum_cm = tc.tile_pool(name="mpsum", bufs=2, space="PSUM")
        mpsum = mpsum_cm.__enter__()

        ft_sb = [tmp_pool.tile([128, 4 * 64], BF16, tag=f"ft{b}", name=f"ft{b}") for b in range(B)]
        gram_sb = tmp_pool.tile([64, B * 64], F32, tag="gram")
        for b in range(B):
            for kk in range(4):
                pst = mpsum.tile([128, 64], BF16, tag="mm", name=f"ft{b}{kk}")
                nc.tensor.transpose(
                    pst[:],
                    c1_own_bf[b][:, 128 * kk : 128 * (kk + 1)],
                    ident[:],
                )
                nc.vector.tensor_copy(
                    ft_sb[b][:, 64 * kk : 64 * (kk + 1)], pst[:, 0:64]
                )
            psg = mpsum.tile([64, 64], F32, tag="mm", name=f"gram{b}")
            for kk in range(4):
                nc.tensor.matmul(
                    psg[:],
                    lhsT=ft_sb[b][:, 64 * kk : 64 * (kk + 1)],
                    rhs=ft_sb[b][:, 64 * kk : 64 * (kk + 1)],
                    start=(kk == 0),
                    stop=(kk == 3),
                )
            nc.vector.tensor_copy(gram_sb[:, 64 * b : 64 * (b + 1)], psg[:])

        # write AG2 contribution: s1 (slab-major) + gram
        for b in range(B):
            nc.sync.dma_start(
                out=bass.AP(
                    tensor=ag2_in[:].tensor,
                    offset=b * C * SLAB,
                    ap=[[SLAB, 64], [B * C * SLAB, 2], [1, SLAB]],
                ),
                in_=s1_own[b][0:64, :].rearrange("p (j s) -> p j s", j=2),
            )
        nc.sync.dma_start(
            out=bass.AP(
                tensor=ag2_in[:].tensor,
                offset=AG2_S1,
                ap=[[64, 64], [64 * 64, B], [1, 64]],
            ),
            in_=gram_sb[:].rearrange("p (b c) -> p b c", b=B),
        )
        nc.gpsimd.collective_compute(
            "AllGather",
            ALU.bypass,
            replica_groups=rg,
            ins=[ag2_in[:].opt()],
            outs=[ag2_out[:].opt()],
        )

        # =========== phase 3: q (local), then k/vT from gathered s1 ===========
        for b in range(B):
            psq = mpsum.tile([64, SHARD], F32, tag="qk", name=f"q{b}")
            nc.tensor.matmul(
                psq[:],
                lhsT=qw_sb[:],
                rhs=s1_own_bf[b][:],
                start=True,
                stop=True,
            )
            nc.vector.tensor_copy(q_stack[64 * b : 64 * (b + 1), :], psq[:])

        # load gathered s1 into s1_pam (global n order); one DMA per slab half
        for b in range(B):
            for j in range(2):
                nc.gpsimd.dma_start(
                    out=s1_pam[b][0:64, :]
                    .rearrange("p (g s) -> p g s", s=2 * SLAB)[:, :, j * SLAB : (j + 1) * SLAB],
                    in_=bass.AP(
                        tensor=ag2_out[:].tensor,
                        offset=b * C * SLAB + j * B * C * SLAB,
                        ap=[[SLAB, 64], [AG2_PER, NCORES], [1, SLAB]],
                    ),
                )
        # gathered gram -> reduce over cores
        gram_full = [tmp_pool.tile([64, 64], F32, tag=f"gramf{b}", name=f"gramf{b}") for b in range(B)]
        for b in range(B):
            gstage = tmp_pool.tile(
                [64, 64, NCORES], F32, tag="gstage", name=f"gstage{b}"
            )
            nc.sync.dma_start(
                out=gstage[:],
                in_=bass.AP(
                    tensor=ag2_out[:].tensor,
                    offset=AG2_S1 + b * C * C,
                    ap=[[64, 64], [1, 64], [AG2_PER, NCORES]],
                ),
            )
            nc.vector.tensor_reduce(gram_full[b][:], gstage[:], axis=AX.X, op=ALU.add)

        for b in range(B):
            for nt in range(8):
                psk = mpsum.tile([64, 512], F32, tag="qk", name=f"k{b}{nt}")
                nc.tensor.matmul(
                    psk[:],
                    lhsT=kw_sb[:],
                    rhs=s1_pam[b][:, 512 * nt : 512 * (nt + 1)],
                    start=True,
                    stop=True,
                )
                nc.vector.tensor_copy(
                    k_stack[64 * b : 64 * (b + 1), 512 * nt : 512 * (nt + 1)],
                    psk[:],
                )
            for mt in range(32):
                psv = mpsum.tile([128, 66], F32, tag="vt", name=f"v{b}{mt}")
                nc.tensor.matmul(
                    psv[:],
                    lhsT=s1_pam[b][:, 128 * mt : 128 * (mt + 1)],
                    rhs=vw_sb[:],
                    start=True,
                    stop=True,
                )
                nc.vector.tensor_copy(
                    vt_sb[b][:, 66 * mt : 66 * (mt + 1)], psv[:]
                )

        # =========== phase 4: CAM finish -> c2 -> pair halo AG ===========
        c2both = acts.tile([128, SHARD], BF16)
        for b in range(B):
            rowmax = tmp_pool.tile([64, 1], F32, tag="camx", name=f"camx{b}")
            den = tmp_pool.tile([64, 1], F32, tag="camd", name=f"camd{b}")
            attn = tmp_pool.tile([64, 64], F32, tag="cama", name=f"cama{b}")
            nc.vector.tensor_reduce(
                rowmax[:], gram_full[b][:], axis=AX.X, op=ALU.min
            )
            nc.scalar.activation(
                attn[:],
                gram_full[b][:],
                AF.Exp,
                bias=rowmax[:],
                scale=-1.0,
                accum_out=den[:],
            )
            nc.vector.reciprocal(den[:], den[:])
            nc.vector.tensor_scalar(attn[:], attn[:], den[:], None, ALU.mult)
            # attn^T via PE
            psat = mpsum.tile([64, 64], F32, tag="mm", name=f"at{b}")
            nc.tensor.transpose(psat[:], attn[:], ident_f32[:])
            attnT = tmp_pool.tile([64, 64], BF16, tag="camat", name=f"camat{b}")
            nc.vector.tensor_copy(attnT[:], psat[:])
            # cam_out = attnT.T @ c1_own
            psco = mpsum.tile([64, SHARD], F32, tag="qk", name=f"co{b}")
            nc.tensor.matmul(
                psco[:],
                lhsT=attnT[:],
                rhs=c1_own_bf[b][:],
                start=True,
                stop=True,
            )
            c2t = tmp_pool.tile([64, SHARD], F32, tag="c2t", name=f"c2t{b}")
            nc.vector.tensor_scalar(c2t[:], psco[:], bnp[:, 11:12], None, ALU.mult)
            nc.vector.tensor_add(
                c2both[64 * b : 64 * (b + 1), :], c2t[:], c1_own[b][:]
            )
            nc.sync.dma_start(
                out=bass.AP(
                    tensor=c2_in[:].tensor,
                    offset=b * C * SLAB,
                    ap=[[SLAB, 64], [B * C * SLAB, 2], [1, SLAB]],
                ),
                in_=c2both[64 * b : 64 * (b + 1), :].rearrange(
                    "p (j s) -> p j s", j=2
                ),
            )
        halo_exchange(c2_in, c2_ag, c2_ri, c2_ro, C)

        mpsum_cm.__exit__(None, None, None)

        # =========== phase 5: PAM attention ===========
        epsum_cm = tc.tile_pool(name="epsum", bufs=3, space="PSUM")
        epsum = epsum_cm.__enter__()
        opsum_cm = tc.tile_pool(name="opsum", bufs=1, space="PSUM")
        opsum = opsum_cm.__enter__()
        apool_cm = tc.tile_pool(name="apool", bufs=3)
        apool = apool_cm.__enter__()

        o_ps = [
            opsum.tile([65, SHARD], F32, name=f"ops{b}", tag=f"ops{b}")
            for b in range(B)
        ]
        for g2 in range(16):
            for b in range(B):
                e_ps = epsum.tile([128, 1024], F32, tag="eg", name=f"e{g2}{b}")
                for j in range(2):
                    mt = 2 * g2 + j
                    nc.tensor.matmul(
                        e_ps[:, 512 * j : 512 * (j + 1)],
                        lhsT=k_stack[
                            64 * b : 64 * (b + 1), 128 * mt : 128 * (mt + 1)
                        ],
                        rhs=q_stack[64 * b : 64 * (b + 1), :],
                        start=True,
                        stop=True,
                        tile_position=(64 * b, 0),
                    )
                a_sb = apool.tile([128, 1024], BF16, tag="ag", name=f"a{g2}{b}")
                nc.scalar.activation(a_sb[:], e_ps[:], AF.Exp)
                for j in range(2):
                    mt = 2 * g2 + j
                    nc.tensor.matmul(
                        o_ps[b][:],
                        lhsT=vt_sb[b][:, 66 * mt : 66 * mt + 65],
                        rhs=a_sb[:, 512 * j : 512 * (j + 1)],
                        start=(mt == 0),
                        stop=(mt == 31),
                    )

        # =========== phase 6: PAM finalize -> s2 -> pair halo AG ===========
        s2both = acts.tile([128, SHARD], BF16)
        for b in range(B):
            recip = tmp_pool.tile([1, SHARD], F32, tag="rec", name=f"rec{b}")
            recipg = tmp_pool.tile([1, SHARD], F32, tag="recg", name=f"recg{b}")
            nc.vector.reciprocal(recip[:], o_ps[b][64:65, :])
            nc.vector.tensor_scalar(
                recipg[:], recip[:], bnp[0:1, 10:11], None, ALU.mult
            )
            nc.sync.dma_start(out=bcast_dram[b : b + 1, :], in_=recipg[:])
            bc_sb = tmp_pool.tile([64, SHARD], F32, tag="bcs", name=f"bcs{b}")
            nc.sync.dma_start(
                out=bc_sb[:],
                in_=bass.AP(
                    tensor=bcast_dram[:].tensor,
                    offset=b * SHARD,
                    ap=[[0, 64], [1, SHARD]],
                ),
            )
            s2t = tmp_pool.tile([64, SHARD], F32, tag="s2t", name=f"s2t{b}")
            nc.vector.tensor_mul(s2t[:], o_ps[b][0:64, :], bc_sb[:])
            nc.vector.tensor_add(
                s2both[64 * b : 64 * (b + 1), :], s2t[:], s1_own[b][0:64, :]
            )
            nc.sync.dma_start(
                out=bass.AP(
                    tensor=s2_in[:].tensor,
                    offset=b * C * SLAB,
                    ap=[[SLAB, 64], [B * C * SLAB, 2], [1, SLAB]],
                ),
                in_=s2both[64 * b : 64 * (b + 1), :].rearrange(
                    "p (j s) -> p j s", j=2
                ),
            )
        halo_exchange(s2_in, s2_ag, s2_ri, s2_ro, C)

        for p in (apool_cm, opsum_cm, epsum_cm):
            p.__exit__(None, None, None)
        cpsum_cm = tc.tile_pool(name="cpsum2", bufs=2, space="PSUM")
        cpsum = cpsum_cm.__enter__()

        # =========== phase 7: conv C1 (on gathered c2) ===========
        c2_loc = acts.tile([128, LOCVIEW], BF16)
        nc.gpsimd.memset(c2_loc[:], 0.0)
        build_view(
            c2_ro, C, None, c2_loc,
            c2both[:].rearrange("p (j w d) -> p j w d", j=2, w=16), "c2",
        )
        wc1_sb = load_wconv(3, "wsC1")
        tC1, statC1 = conv64(wc1_sb, c2_loc, cpsum, "cC1")

        # =========== phase 8: conv S1 (on gathered s2) ===========
        s2_loc = acts.tile([128, LOCVIEW], BF16)
        nc.gpsimd.memset(s2_loc[:], 0.0)
        build_view(
            s2_ro, C, None, s2_loc,
            s2both[:].rearrange("p (j w d) -> p j w d", j=2, w=16), "s2",
        )
        ws1_sb = load_wconv(2, "wsS1")
        tS1, statS1 = conv64(ws1_sb, s2_loc, cpsum, "cS1")

        st2_sb = stats_pool.tile([64, 4], F32)
        pack_stats(st2_sb, [statS1, statC1])
        nc.sync.dma_start(out=st2_in[:], in_=st2_sb[:])
        nc.gpsimd.collective_compute(
            "AllGather",
            ALU.bypass,
            replica_groups=rg,
            ins=[st2_in[:].opt()],
            outs=[st2_out[:].opt()],
        )
        st2_stage = stats_pool.tile([64, 4, NCORES], F32)
        nc.sync.dma_start(
            out=st2_stage[:],
            in_=bass.AP(
                tensor=st2_out[:].tensor,
                offset=0,
                ap=[[4, 64], [1, 4], [256, NCORES]],
            ),
        )
        st2_tot = stats_pool.tile([64, 4], F32)
        nc.vector.tensor_reduce(st2_tot[:], st2_stage[:], axis=AX.X, op=ALU.add)
        cS1 = bn_coeffs(st2_tot, 0, bnp[:, 4:5], bnp[:, 5:6], "bnS1")
        cC1 = bn_coeffs(st2_tot, 2, bnp[:, 6:7], bnp[:, 7:8], "bnC1")

        fcat_own = acts.tile([128, B * SHARD], BF16)
        for b in range(B):
            bn_rrelu(tS1[b], cS1, fcat_own[0:64, b * SHARD : (b + 1) * SHARD])
            bn_rrelu(tC1[b], cC1, fcat_own[64:128, b * SHARD : (b + 1) * SHARD])

        # contribution: (2 slab, 2 b, 128 c, 256); one DMA per batch
        for b in range(B):
            nc.sync.dma_start(
                out=bass.AP(
                    tensor=fc_in[:].tensor,
                    offset=b * 2 * C * SLAB,
                    ap=[[SLAB, 128], [B * 2 * C * SLAB, 2], [1, SLAB]],
                ),
                in_=fcat_own[:, b * SHARD : (b + 1) * SHARD].rearrange(
                    "p (j s) -> p j s", j=2
                ),
            )
        halo_exchange(fc_in, fc_ag, fc_ri, fc_ro, 2 * C)

        # =========== phase 9: conv F ===========
        fcat_loc = [acts.tile([128, LOCVIEW], BF16, name=f"fl{b}") for b in range(B)]
        for b in range(B):
            nc.gpsimd.memset(fcat_loc[b][:], 0.0)
            build_view(
                fc_ro, 2 * C, b, fcat_loc[b],
                fcat_own[:, b * SHARD : (b + 1) * SHARD].rearrange(
                    "p (j w d) -> p j w d", j=2, w=16
                ),
                f"fc{b}",
            )
        wf_sb = load_wconv(4, "wsF")
        tF, statF = conv128(wf_sb, fcat_loc, cpsum, "cF")

        stf_sb = stats_pool.tile([64, 2], F32)
        pack_stats(stf_sb, [statF])
        nc.sync.dma_start(out=stf_in[:], in_=stf_sb[:])
        nc.gpsimd.collective_compute(
            "AllGather",
            ALU.bypass,
            replica_groups=rg,
            ins=[stf_in[:].opt()],
            outs=[stf_out[:].opt()],
        )
        stf_stage = stats_pool.tile([64, 2, NCORES], F32)
        nc.sync.dma_start(
            out=stf_stage[:],
            in_=bass.AP(
                tensor=stf_out[:].tensor,
                offset=0,
                ap=[[2, 64], [1, 2], [128, NCORES]],
            ),
        )
        stf_tot = stats_pool.tile([64, 2], F32)
        nc.vector.tensor_reduce(stf_tot[:], stf_stage[:], axis=AX.X, op=ALU.add)
        cF = bn_coeffs(stf_tot, 0, bnp[:, 8:9], bnp[:, 9:10], "bnF")

        out_own = acts.tile([128, SHARD], BF16)
        for b in range(B):
            bn_rrelu(tF[b], cF, out_own[64 * b : 64 * (b + 1), :])
        # split the output into hi/lo byte planes (wire compresses hi)
        ohi16 = acts.tile([128, SHARD], U16)
        olo16 = acts.tile([128, SHARD], U16)
        ohi8 = acts.tile([128, SHARD], U8)
        olo8 = acts.tile([128, SHARD], U8)
        nc.vector.tensor_scalar(
            ohi16[:], out_own[:].bitcast(U16), 8, None, ALU.logical_shift_right
        )
        nc.vector.tensor_scalar(
            olo16[:], out_own[:].bitcast(U16), 255, None, ALU.bitwise_and
        )
        nc.vector.tensor_copy(ohi8[:], ohi16[:])
        nc.vector.tensor_copy(olo8[:], olo16[:])
        half = B * C * SHARD // 2  # carrier elems per plane
        nc.sync.dma_start(
            out=bass.AP(
                tensor=out_d,
                offset=0,
                ap=[[SHARD // 2, 128], [1, SHARD // 2]],
            ).bitcast(U8),
            in_=ohi8[:],
        )
        nc.sync.dma_start(
            out=bass.AP(
                tensor=out_d,
                offset=half,
                ap=[[SHARD // 2, 128], [1, SHARD // 2]],
            ).bitcast(U8),
            in_=olo8[:],
        )

        for p in (cpsum_cm, tmp_pool_cm, stats_pool_cm, wpool_cm, acts_cm,
                  singles_cm, dram_cm):
            p.__exit__(None, None, None)

    nc.finalize()
    return nc


def _prep_host(inputs):
    """Build per-core in_maps from the full problem inputs."""
    import ml_dtypes

    x = np.asarray(inputs["x"], np.float32)

    # x: own 2 slabs per core, compact (2, B, C, 256); halos move on-device
    xs = x.reshape(B, C, HH, SLAB).transpose(2, 0, 1, 3)  # (16 slab, B, C, 256)
    xs_bf = np.ascontiguousarray(xs).astype(ml_dtypes.bfloat16)

    # weight blob: [392, 1728]
    blob = np.zeros((WB_ROWS, WB_W), np.float32)

    def rows(w):
        # (O, I, 3, 3, 3) -> (I, 27*64): row = input channel, col = (off, o)
        w = np.asarray(w, np.float32)
        return np.transpose(w, (1, 2, 3, 4, 0)).reshape(w.shape[1], WB_W)

    blob[0:64] = rows(inputs["wS"])
    blob[64:128] = rows(inputs["wC"])
    blob[128:192] = rows(inputs["wS1"])
    blob[192:256] = rows(inputs["wC1"])
    blob[256:384] = rows(inputs["wF"])

    qw = np.asarray(inputs["qw"], np.float32).reshape(64, 64)
    kw = np.asarray(inputs["kw"], np.float32).reshape(64, 64)
    vw = np.asarray(inputs["vw"], np.float32).reshape(64, 64)
    qa = np.zeros((65, 64), np.float32)
    qa[:64] = qw.T
    qa[64] = np.asarray(inputs["qb"], np.float32)
    ka = np.zeros((65, 64), np.float32)
    ka[:64] = kw.T
    ka[64] = np.asarray(inputs["kb"], np.float32)
    va = np.zeros((65, 66), np.float32)
    va[:64, :64] = vw.T
    va[64, :64] = np.asarray(inputs["vb"], np.float32)
    va[64, 64] = 1.0
    flat = blob.reshape(-1)
    flat[QKV_OFF : QKV_OFF + 65 * 64] = qa.reshape(-1)
    flat[QKV_OFF + 65 * 64 : QKV_OFF + 2 * 65 * 64] = ka.reshape(-1)
    flat[QKV_OFF + 2 * 65 * 64 : QKV_OFF + 2 * 65 * 64 + 65 * 66] = va.reshape(-1)
    blob_bf = blob.astype(ml_dtypes.bfloat16).reshape(NCORES, WB_SH, WB_W)

    bnp = np.zeros((64, 12), np.float32)
    for ci, k in enumerate(
        ("gS", "bS", "gC", "bC", "gS1", "bS1", "gC1", "bC1", "gF", "bF")
    ):
        bnp[:, ci] = np.asarray(inputs[k], np.float32)
    bnp[:, 10] = float(np.asarray(inputs["gamma_p"]).reshape(-1)[0])
    bnp[:, 11] = float(np.asarray(inputs["gamma_c"]).reshape(-1)[0])
    bnp_bf = bnp.astype(ml_dtypes.bfloat16).reshape(-1)

    in_maps = []
    for i in range(NCORES):
        # x in (b,c) x (j,s) order to match the on-device SBUF layout
        x2c = np.ascontiguousarray(
            xs_bf[2 * i : 2 * i + 2].transpose(1, 2, 0, 3)
        ).reshape(-1)
        xb = x2c.view(np.uint8).reshape(-1, 2)
        wb = blob_bf[i].reshape(-1).view(np.uint8).reshape(-1, 2)
        core_bytes = np.concatenate(
            [
                np.ascontiguousarray(xb[:, 1]),  # x hi plane
                np.ascontiguousarray(xb[:, 0]),  # x lo plane
                np.ascontiguousarray(wb[:, 1]),  # w hi plane
                np.ascontiguousarray(wb[:, 0]),  # w lo plane
                bnp_bf.view(np.uint8),
            ]
        )
        in_maps.append({"blob": core_bytes.view(ml_dtypes.bfloat16)})
    return in_maps


_PROG_CACHE = {}


def kernel(**inputs) -> np.ndarray:
    if "nc" not in _PROG_CACHE:
        _PROG_CACHE["nc"] = build_program()
    nc = _PROG_CACHE["nc"]
    in_maps = _prep_host(inputs)
    res = run_bass_kernel_spmd(nc, in_maps, list(range(NCORES))).results
    out = np.zeros((B, C, HH, HH, HH), np.float32)
    ov = out.reshape(B, C, NCORES, 2, SLAB)
    for i in range(NCORES):
        planes = np.asarray(res[i]["out"]).view(np.uint8).reshape(2, -1)
        bits = (planes[0].astype(np.uint16) << 8) | planes[1]
        vals = bits.view(ml_dtypes_bf16()).astype(np.float32)
        ov[:, :, i] = vals.reshape(B, C, 2, SLAB)
    return out


def ml_dtypes_bf16():
    import ml_dtypes

    return ml_dtypes.bfloat16


if __name__ == "__main__":
    rng = np.random.default_rng(0)
    print("building program...")
    nc = build_program()
    print("ok")

